# revision 26
# baseline (speedup 1.0000x reference)
"""Block-circulant linear layer (CirculantLinear) as a Trainium2 Bass kernel.

Frequency-domain formulation: per rfft bin k, a dense [128x -> 128y] complex
matmul F_out[b,y,k] = sum_x F_e[y,x,k] * F_x[b,x,k].  Bins 0,4 are purely
real -> 14 real [128,128] @ [128,4096] matmuls per core.  The length-8
rfft of x and irfft of the result are done on the host.

Precision (gate 2e-2): input planes fp8 e3m4, weights bf16, fp32 PSUM,
ALL 8 output planes e3m4 scaled by 1/2 (folded into W).  Measured
~1.9e-2 total.

HBM traffic per core: 4.19 MB in + 0.45 MB W + 4.19 MB out = 8.84 MB
(24.7 us at the 358 GB/s per-core HBM roofline).  PE: 112 matmuls of
[128,128]x[128,512] at 216 ns = 24.2 us.  PE and HBM co-limit.

DMA layout: one interleaved input tensor fx [128, 8*4096] (column block
p = plane p, in compute-consumption order) so transfers use 4-8 KB
descriptors; plane pairs that feed the same bin share an 8 KB-row
transfer.  W slots are ordered by compute order and split into two
pieces that land just ahead of their groups.  All triggers issue from
SP in program order (inputs first => input-priority FIFO on the qSP
HWDGE ring); trigger cost scales with descriptor count (~5 ns/desc),
so the big-descriptor layout also shrinks serial trigger time ~4x.

Timeline on HW (full-clock run): ~6.8 us fixed runtime preamble before
any user instruction; SP triggers from ~7.2; first descriptors execute
~8.8; plane 0's completion semaphore ~11.7; 10 warmup matmuls bridge
the PE from ~8.0 to that point (a PE gap resets the HAM p-state: the
clock drops to ~1.2 GHz and takes ~3 us of continuous work to come
back).  Re4 runs mid-schedule so the kernel ends on a complex group,
whose 3.5 us window lets ACT/DVE drain the eviction queue before the
last PSUM tile -- ending on a real group leaves ~3 us of eviction
backlog after the last matmul.

Output: one fo [128, 8*4096] e3m4 tensor, flushed per plane as each
finishes evicting (last plane in halves to shrink the tail: final
transfer 256 KB + ~1.4 us completion latency + ~2.3 us epilogue).
"""

import sys

import numpy as np

_TRN = "/opt/trn_rl_repo"
if _TRN not in sys.path:
    sys.path.insert(0, _TRN)

# If the image's antenv lacks axon_hooks, stub it so bass_utils' trace
# path (taken when BASS_TRACE=1 is set in the environment) cannot crash.
try:
    import antenv.axon_hooks  # noqa: F401
except Exception:  # pragma: no cover
    import types

    _m = types.ModuleType("antenv.axon_hooks")
    _m._hook = None
    _m.set_axon_ntff_profile_hook = lambda h: setattr(_m, "_hook", h)
    _m.get_axon_ntff_profile_hook = lambda: getattr(_m, "_hook", None)
    sys.modules["antenv.axon_hooks"] = _m

import ml_dtypes

import concourse.bacc as bacc
import concourse.bass as bass
import concourse.mybir as mybir
from concourse.bass_utils import run_bass_kernel_spmd
from concourse.tile import TileContext

_dt = mybir.dt
_bf16 = np.dtype(ml_dtypes.bfloat16)
_f8 = np.dtype(ml_dtypes.float8_e3m4)
_CLIP = 15.0   # e3m4 max finite is 15.5; clip inputs so the cast can't overflow
_OSC = 0.5     # scale folded into every W slot; outputs stay well inside e3m4

N_CORES = 8
B, IN_CH, OUT_CH, MINI = 32768, 1024, 1024, 8
GY, GX = OUT_CH // MINI, IN_CH // MINI  # 128, 128
P = 128
BS = B // N_CORES  # rows per core (4096)
NS = 512           # batch columns per matmul (one PSUM bank)
NU = 4             # PSUM tiles per out-plane ([128, 1024] each)
NPL = 8            # planes, in compute-consumption order
NW = 14

# input plane order (DRAM column blocks): Xr0, Xr1, Xi1, Xr2, Xi2, Xr3, Xi3, Xr4
# rfft bin -> (re plane idx, im plane idx or None)
_PL_RE = {0: 0, 1: 1, 2: 3, 3: 5, 4: 7}
_PL_IM = {1: 2, 2: 4, 3: 6}

# The W matrix rides INSIDE the fx tensor as raw bytes (device reads it
# back through a bf16 bitcast view), interleaved with the planes in
# consumption order.  W+plane0 then share ONE transfer (one ~0.65us SP
# trigger instead of two, 4.75KB descriptors), and every later piece
# lands just ahead of the group that consumes it.  Byte-column offsets:
_WA_OFF, _WA_B = 0, 3 * 2 * P        # slots 0-2   (Re0, Re1)
_WB_OFF, _WB_B = 13056, 11 * 2 * P   # slots 3-13
_XOFF = {0: 768, 1: 4864, 2: 8960, 3: 15872, 4: 19968,
         5: 28160, 6: 32256, 7: 24064}
_TOT = 36352
# transfer boundaries (byte cols), in SP-ring order
_XFERS = [(0, 4864), (4864, 8960), (8960, 13056),
          (13056, 19968), (19968, 28160), (28160, 36352)]

# groups in compute order: (kind, w slots, x planes)
# out plane g is group g's result; out DRAM column block = g
# Re4 sits mid-schedule: real groups produce 8 evict-ops in half the PE
# time of complex ones, so ending on a complex group lets ACT/DVE drain
# their queues before the last eviction -> ~2us less tail.
# W slot layout (wd columns) follows group order:
#   0: Wr0/2 | 1,2: Re1 [Wr/2,-Wi/2] | 3,4: Im1 [Wi/2,Wr/2] | 5,6: Re2
#   | 7: Wr4/2 | 8,9: Im2 | 10,11: Re3 | 12,13: Im3
_GROUPS = [
    ("real", (0,), (0,)),        # Re0
    ("cplx", (1, 2), (1, 2)),    # Re1
    ("cplx", (3, 4), (1, 2)),    # Im1
    ("cplx", (5, 6), (3, 4)),    # Re2
    ("real", (7,), (7,)),        # Re4
    ("cplx", (8, 9), (3, 4)),    # Im2
    ("cplx", (10, 11), (5, 6)),  # Re3
    ("cplx", (12, 13), (5, 6)),  # Im3
]


def _build_nc(bs: int = BS) -> bass.Bass:
    nc = bacc.Bacc()
    fx_d = nc.declare_dram_parameter("fx", [P, _TOT], _dt.float8e3, isOutput=False)
    fo_d = nc.declare_dram_parameter("fo", [P, NPL * bs], _dt.float8e3, isOutput=True)

    with TileContext(nc) as tc:
        with (
            tc.tile_pool(name="wpool", bufs=1) as wpool,
            tc.tile_pool(name="xpool", bufs=1) as xpool,
            tc.tile_pool(name="opool", bufs=1) as opool,
            tc.tile_pool(name="pso", bufs=1, space="PSUM") as pso,
        ):
            xall = xpool.tile([P, _TOT], _dt.float8e3, name="xall")

            # ---- input transfers, SP ring, in consumption order ----
            # W rides inside fx as bytes; transfer 0 = w_a + plane 0 (one
            # trigger gates the first matmul), then planes/W pieces in the
            # order the groups consume them.  All on the SP HWDGE ring:
            # FIFO = input priority, and later output flushes queue
            # strictly behind the inputs.
            for a, b in _XFERS:
                nc.sync.dma_start(out=xall[:, a:b], in_=fx_d[:, a:b])

            def xp(pl):
                o = _XOFF[pl]
                return xall[:, o : o + bs]

            # PE p-state warmup on a zeroed scratch tile while the first
            # input transfer streams in (results discarded; real groups
            # reset PSUM with start=True).
            warm = wpool.tile([P, 5 * P], _dt.bfloat16, name="warm")
            nc.gpsimd.memzero(warm[:])
            wps = pso.tile([P, 2 * NS], _dt.float32, tag="u0", name="wps")
            # Warmups must bridge the PE from its main start (~8us) all
            # the way to transfer 0's completion semaphore (~11.4us): a
            # gap resets the HAM p-state -> ~3us of half-clock after.
            for _ in range(8):
                nc.tensor.matmul(
                    wps[:, 0:NS],
                    lhsT=warm[:, 0:P],
                    rhs=warm[:, P:],
                    start=True,
                    stop=True,
                )

            op = [
                opool.tile([P, bs], _dt.float8e3, tag=f"o{g}", name=f"op{g}")
                for g in range(NPL)
            ]

            def w_slot(i):
                # bf16 view of the W bytes embedded in the fp8 fx tile
                if i < 3:
                    o = _WA_OFF + i * 2 * P
                else:
                    o = _WB_OFF + (i - 3) * 2 * P
                return xall[:, o : o + 2 * P].bitcast(_dt.bfloat16)

            def evict(ps, g, u):
                # split each 2-bank eviction across ACT and DVE; flush the
                # finished plane over the SP ring (queues behind inputs =>
                # input-priority; last plane in halves to shrink the tail)
                c0 = u * 2 * NS
                nc.scalar.copy(op[g][:, c0 : c0 + NS], ps[:, 0:NS])
                nc.vector.tensor_copy(op[g][:, c0 + NS : c0 + 2 * NS], ps[:, NS:])
                base = g * bs
                if g == NPL - 1:
                    # flush the last plane in halves so the final
                    # transfer (and its completion latency) is small
                    if u == 1:
                        nc.sync.dma_start(
                            out=fo_d[:, base : base + bs // 2],
                            in_=op[g][:, 0 : bs // 2],
                        )
                    elif u == NU - 1:
                        nc.sync.dma_start(
                            out=fo_d[:, base + bs // 2 : base + bs],
                            in_=op[g][:, bs // 2 :],
                        )
                elif u == NU - 1:
                    nc.sync.dma_start(
                        out=fo_d[:, base : base + bs], in_=op[g][:]
                    )

            def mm(ps, slot, pl, s, start, stop):
                nc.tensor.matmul(
                    ps[:, (s % 2) * NS : (s % 2 + 1) * NS],
                    lhsT=w_slot(slot),
                    rhs=xp(pl)[:, s * NS : (s + 1) * NS],
                    start=start,
                    stop=stop,
                )

            for g, (kind, slots, planes) in enumerate(_GROUPS):
                if kind == "real":
                    for u in range(NU):
                        ps = pso.tile(
                            [P, 2 * NS], _dt.float32, tag=f"u{u}", name=f"ps_{g}_{u}"
                        )
                        mm(ps, slots[0], planes[0], 2 * u, True, True)
                        mm(ps, slots[0], planes[0], 2 * u + 1, True, True)
                        evict(ps, g, u)
                else:
                    tiles = []
                    for u in range(NU):
                        ps = pso.tile(
                            [P, 2 * NS], _dt.float32, tag=f"u{u}", name=f"ps_{g}_{u}"
                        )
                        tiles.append(ps)
                        mm(ps, slots[0], planes[0], 2 * u, True, False)
                        mm(ps, slots[0], planes[0], 2 * u + 1, True, False)
                    for u in range(NU):
                        mm(tiles[u], slots[1], planes[1], 2 * u, False, True)
                        mm(tiles[u], slots[1], planes[1], 2 * u + 1, False, True)
                        evict(tiles[u], g, u)
    nc.compile()
    return nc


def _host_pack(x: np.ndarray, eigens: np.ndarray):
    """Build the interleaved per-core fx planes and the W block."""
    xb = np.ascontiguousarray(x, dtype=np.float32).reshape(B, GX, MINI)
    Fx = np.fft.rfft(xb, axis=-1)  # [B, 128, 5] complex64

    planes = np.empty((P, NPL, B), dtype=_f8)
    for k, pl in _PL_RE.items():
        planes[:, pl, :] = np.clip(Fx[:, :, k].real.T, -_CLIP, _CLIP).astype(_f8)
    for k, pl in _PL_IM.items():
        planes[:, pl, :] = np.clip(Fx[:, :, k].imag.T, -_CLIP, _CLIP).astype(_f8)

    Fe = np.fft.fft(eigens.astype(np.complex64), axis=-1)  # [y, x, 8]
    M = [Fe[:, :, k].T for k in range(5)]  # M_k[x, y]
    # slot pairs in group order: Re_k -> [Wr/2, -Wi/2]; Im_k -> [Wi/2, Wr/2]
    slots = [M[0].real]
    for k, part in ((1, "re"), (1, "im"), (2, "re")):
        if part == "re":
            slots += [M[k].real, -M[k].imag]
        else:
            slots += [M[k].imag, M[k].real]
    slots += [M[4].real]
    for k, part in ((2, "im"), (3, "re"), (3, "im")):
        if part == "re":
            slots += [M[k].real, -M[k].imag]
        else:
            slots += [M[k].imag, M[k].real]
    wd = np.empty((P, NW * P), dtype=np.float32)
    for i, s in enumerate(slots):
        wd[:, i * P : (i + 1) * P] = s * _OSC
    return planes, np.ascontiguousarray(wd.astype(_bf16)).view(np.uint8)


def _host_unpack(res_list) -> np.ndarray:
    """Per-core fo planes -> full [B, OUT_CH] fp32."""
    out = np.empty((B, OUT_CH), dtype=np.float32)
    inv = 1.0 / _OSC
    for c, r in enumerate(res_list):
        # out plane order (= group order): Re0,Re1,Im1,Re2,Re4,Im2,Re3,Im3
        f = np.asarray(r["fo"]).astype(np.float32).reshape(P, NPL, BS) * inv
        Fo = np.empty((BS, GY, 5), dtype=np.complex64)
        Fo[:, :, 0] = f[:, 0].T
        Fo[:, :, 1] = (f[:, 1] + 1j * f[:, 2]).T
        Fo[:, :, 2] = (f[:, 3] + 1j * f[:, 5]).T
        Fo[:, :, 3] = (f[:, 6] + 1j * f[:, 7]).T
        Fo[:, :, 4] = f[:, 4].T
        blk = np.fft.irfft(Fo, n=MINI, axis=-1).astype(np.float32)
        out[c * BS : (c + 1) * BS] = blk.reshape(BS, OUT_CH)
    return out


def _run(x: np.ndarray, eigens: np.ndarray, trace: bool = False):
    planes, wdb = _host_pack(x, np.asarray(eigens, dtype=np.float32))
    nc = _build_nc()
    in_maps = []
    for i in range(N_CORES):
        buf = np.empty((P, _TOT), dtype=np.uint8)
        buf[:, _WA_OFF : _WA_OFF + _WA_B] = wdb[:, :_WA_B]
        buf[:, _WB_OFF : _WB_OFF + _WB_B] = wdb[:, _WA_B:]
        for pl, off in _XOFF.items():
            buf[:, off : off + BS] = planes[
                :, pl, i * BS : (i + 1) * BS
            ].view(np.uint8)
        in_maps.append({"fx": buf.view(_f8)})
    res = run_bass_kernel_spmd(nc, in_maps, list(range(N_CORES)), trace=trace)
    out = _host_unpack([res.results[i] for i in range(N_CORES)])
    return out, res


def kernel(x: np.ndarray, eigens: np.ndarray) -> np.ndarray:
    out, _ = _run(x, eigens)
    return out


# revision 27
# speedup vs baseline: 1.0331x; 1.0331x over previous
"""Block-circulant linear layer (CirculantLinear) as a Trainium2 Bass kernel.

Frequency-domain formulation: per rfft bin k, a dense [128x -> 128y] complex
matmul F_out[b,y,k] = sum_x F_e[y,x,k] * F_x[b,x,k].  Bins 0,4 are purely
real -> 14 real [128,128] @ [128,4096] matmuls per core.  The length-8
rfft of x and irfft of the result are done on the host.

Precision (gate 2e-2): input planes fp8 e3m4, weights bf16, fp32 PSUM,
ALL 8 output planes e3m4 scaled by 1/2 (folded into W).  Measured
~1.9e-2 total.

HBM traffic per core: 4.19 MB in + 0.45 MB W + 4.19 MB out = 8.84 MB
(24.7 us at the 358 GB/s per-core HBM roofline).  PE: 112 matmuls of
[128,128]x[128,512] at 216 ns = 24.2 us.  PE and HBM co-limit.

DMA layout: one interleaved input tensor fx [128, 8*4096] (column block
p = plane p, in compute-consumption order) so transfers use 4-8 KB
descriptors; plane pairs that feed the same bin share an 8 KB-row
transfer.  W slots are ordered by compute order and split into two
pieces that land just ahead of their groups.  All triggers issue from
SP in program order (inputs first => input-priority FIFO on the qSP
HWDGE ring); trigger cost scales with descriptor count (~5 ns/desc),
so the big-descriptor layout also shrinks serial trigger time ~4x.

Timeline on HW (full-clock run): ~6.8 us fixed runtime preamble before
any user instruction; SP triggers from ~7.2; first descriptors execute
~8.8; plane 0's completion semaphore ~11.7; 10 warmup matmuls bridge
the PE from ~8.0 to that point (a PE gap resets the HAM p-state: the
clock drops to ~1.2 GHz and takes ~3 us of continuous work to come
back).  Re4 runs mid-schedule so the kernel ends on a complex group,
whose 3.5 us window lets ACT/DVE drain the eviction queue before the
last PSUM tile -- ending on a real group leaves ~3 us of eviction
backlog after the last matmul.

Output: one fo [128, 8*4096] e3m4 tensor, flushed per plane as each
finishes evicting (last plane in halves to shrink the tail: final
transfer 256 KB + ~1.4 us completion latency + ~2.3 us epilogue).
"""

import sys

import numpy as np

_TRN = "/opt/trn_rl_repo"
if _TRN not in sys.path:
    sys.path.insert(0, _TRN)

# If the image's antenv lacks axon_hooks, stub it so bass_utils' trace
# path (taken when BASS_TRACE=1 is set in the environment) cannot crash.
try:
    import antenv.axon_hooks  # noqa: F401
except Exception:  # pragma: no cover
    import types

    _m = types.ModuleType("antenv.axon_hooks")
    _m._hook = None
    _m.set_axon_ntff_profile_hook = lambda h: setattr(_m, "_hook", h)
    _m.get_axon_ntff_profile_hook = lambda: getattr(_m, "_hook", None)
    sys.modules["antenv.axon_hooks"] = _m

import ml_dtypes

import concourse.bacc as bacc
import concourse.bass as bass
import concourse.mybir as mybir
from concourse.bass_utils import run_bass_kernel_spmd
from concourse.tile import TileContext

_dt = mybir.dt
_bf16 = np.dtype(ml_dtypes.bfloat16)
_f8 = np.dtype(ml_dtypes.float8_e3m4)
_CLIP = 15.0   # e3m4 max finite is 15.5; clip inputs so the cast can't overflow
_OSC = 0.5     # scale folded into every W slot; outputs stay well inside e3m4

N_CORES = 8
B, IN_CH, OUT_CH, MINI = 32768, 1024, 1024, 8
GY, GX = OUT_CH // MINI, IN_CH // MINI  # 128, 128
P = 128
BS = B // N_CORES  # rows per core (4096)
NS = 512           # batch columns per matmul (one PSUM bank)
NU = 4             # PSUM tiles per out-plane ([128, 1024] each)
NPL = 8            # planes, in compute-consumption order
NW = 14

# input plane order (DRAM column blocks): Xr0, Xr1, Xi1, Xr2, Xi2, Xr3, Xi3, Xr4
# rfft bin -> (re plane idx, im plane idx or None)
_PL_RE = {0: 0, 1: 1, 2: 3, 3: 5, 4: 7}
_PL_IM = {1: 2, 2: 4, 3: 6}

# The W matrix rides INSIDE the fx tensor as raw bytes (device reads it
# back through a bf16 bitcast view), interleaved with the planes in
# consumption order.  W+plane0 then share ONE transfer (one ~0.65us SP
# trigger instead of two, 4.75KB descriptors), and every later piece
# lands just ahead of the group that consumes it.  Byte-column offsets:
_WA_OFF, _WA_B = 0, 3 * 2 * P        # slots 0-2   (Re0, Re1)
_WB_OFF, _WB_B = 13056, 11 * 2 * P   # slots 3-13
_XOFF = {0: 768, 1: 4864, 2: 8960, 3: 15872, 4: 19968,
         5: 28160, 6: 32256, 7: 24064}
_TOT = 36352
# transfer boundaries (byte cols), in SP-ring order.  w_b gets its own
# transfer: a semaphore covers its whole transfer, and burying the small
# w_b inside an 864KB transfer made Im1 wait ~0.5us for bytes it
# doesn't read.
_XFERS = [(0, 4864), (4864, 8960), (8960, 13056), (13056, 15872),
          (15872, 19968), (19968, 28160), (28160, 36352)]

# groups in compute order: (kind, w slots, x planes)
# out plane g is group g's result; out DRAM column block = g
# Re4 sits mid-schedule: real groups produce 8 evict-ops in half the PE
# time of complex ones, so ending on a complex group lets ACT/DVE drain
# their queues before the last eviction -> ~2us less tail.
# W slot layout (wd columns) follows group order:
#   0: Wr0/2 | 1,2: Re1 [Wr/2,-Wi/2] | 3,4: Im1 [Wi/2,Wr/2] | 5,6: Re2
#   | 7: Wr4/2 | 8,9: Im2 | 10,11: Re3 | 12,13: Im3
_GROUPS = [
    ("real", (0,), (0,)),        # Re0
    ("cplx", (1, 2), (1, 2)),    # Re1
    ("cplx", (3, 4), (1, 2)),    # Im1
    ("cplx", (5, 6), (3, 4)),    # Re2
    ("real", (7,), (7,)),        # Re4
    ("cplx", (8, 9), (3, 4)),    # Im2
    ("cplx", (10, 11), (5, 6)),  # Re3
    ("cplx", (12, 13), (5, 6)),  # Im3
]


def _build_nc(bs: int = BS) -> bass.Bass:
    nc = bacc.Bacc()
    fx_d = nc.declare_dram_parameter("fx", [P, _TOT], _dt.float8e3, isOutput=False)
    fo_d = nc.declare_dram_parameter("fo", [P, NPL * bs], _dt.float8e3, isOutput=True)

    with TileContext(nc) as tc:
        with (
            tc.tile_pool(name="wpool", bufs=1) as wpool,
            tc.tile_pool(name="xpool", bufs=1) as xpool,
            tc.tile_pool(name="opool", bufs=1) as opool,
            tc.tile_pool(name="pso", bufs=1, space="PSUM") as pso,
        ):
            xall = xpool.tile([P, _TOT], _dt.float8e3, name="xall")

            # ---- input transfers, SP ring, in consumption order ----
            # W rides inside fx as bytes; transfer 0 = w_a + plane 0 (one
            # trigger gates the first matmul), then planes/W pieces in the
            # order the groups consume them.  All on the SP HWDGE ring:
            # FIFO = input priority, and later output flushes queue
            # strictly behind the inputs.
            for a, b in _XFERS:
                nc.sync.dma_start(out=xall[:, a:b], in_=fx_d[:, a:b])

            def xp(pl):
                o = _XOFF[pl]
                return xall[:, o : o + bs]

            # PE p-state warmup on a zeroed scratch tile while the first
            # input transfer streams in (results discarded; real groups
            # reset PSUM with start=True).
            warm = wpool.tile([P, 5 * P], _dt.bfloat16, name="warm")
            nc.gpsimd.memzero(warm[:])
            wps = pso.tile([P, 2 * NS], _dt.float32, tag="u0", name="wps")
            # Warmups must bridge the PE from its main start (~8us) all
            # the way to transfer 0's completion semaphore (~11.4us): a
            # gap resets the HAM p-state -> ~3us of half-clock after.
            for _ in range(8):
                nc.tensor.matmul(
                    wps[:, 0:NS],
                    lhsT=warm[:, 0:P],
                    rhs=warm[:, P:],
                    start=True,
                    stop=True,
                )

            op = [
                opool.tile([P, bs], _dt.float8e3, tag=f"o{g}", name=f"op{g}")
                for g in range(NPL)
            ]

            def w_slot(i):
                # bf16 view of the W bytes embedded in the fp8 fx tile
                if i < 3:
                    o = _WA_OFF + i * 2 * P
                else:
                    o = _WB_OFF + (i - 3) * 2 * P
                return xall[:, o : o + 2 * P].bitcast(_dt.bfloat16)

            def evict(ps, g, u):
                # split each 2-bank eviction across ACT and DVE; flush the
                # finished plane over the SP ring (queues behind inputs =>
                # input-priority; last plane in halves to shrink the tail)
                c0 = u * 2 * NS
                nc.scalar.copy(op[g][:, c0 : c0 + NS], ps[:, 0:NS])
                nc.vector.tensor_copy(op[g][:, c0 + NS : c0 + 2 * NS], ps[:, NS:])
                base = g * bs
                if g == NPL - 1:
                    # flush the last plane in halves so the final
                    # transfer (and its completion latency) is small
                    if u == 1:
                        nc.sync.dma_start(
                            out=fo_d[:, base : base + bs // 2],
                            in_=op[g][:, 0 : bs // 2],
                        )
                    elif u == NU - 1:
                        nc.sync.dma_start(
                            out=fo_d[:, base + bs // 2 : base + bs],
                            in_=op[g][:, bs // 2 :],
                        )
                elif u == NU - 1:
                    nc.sync.dma_start(
                        out=fo_d[:, base : base + bs], in_=op[g][:]
                    )

            def mm(ps, slot, pl, s, start, stop):
                nc.tensor.matmul(
                    ps[:, (s % 2) * NS : (s % 2 + 1) * NS],
                    lhsT=w_slot(slot),
                    rhs=xp(pl)[:, s * NS : (s + 1) * NS],
                    start=start,
                    stop=stop,
                )

            for g, (kind, slots, planes) in enumerate(_GROUPS):
                if kind == "real":
                    for u in range(NU):
                        ps = pso.tile(
                            [P, 2 * NS], _dt.float32, tag=f"u{u}", name=f"ps_{g}_{u}"
                        )
                        mm(ps, slots[0], planes[0], 2 * u, True, True)
                        mm(ps, slots[0], planes[0], 2 * u + 1, True, True)
                        evict(ps, g, u)
                else:
                    tiles = []
                    for u in range(NU):
                        ps = pso.tile(
                            [P, 2 * NS], _dt.float32, tag=f"u{u}", name=f"ps_{g}_{u}"
                        )
                        tiles.append(ps)
                        mm(ps, slots[0], planes[0], 2 * u, True, False)
                        mm(ps, slots[0], planes[0], 2 * u + 1, True, False)
                    for u in range(NU):
                        mm(tiles[u], slots[1], planes[1], 2 * u, False, True)
                        mm(tiles[u], slots[1], planes[1], 2 * u + 1, False, True)
                        evict(tiles[u], g, u)
    nc.compile()
    return nc


def _host_pack(x: np.ndarray, eigens: np.ndarray):
    """Build the interleaved per-core fx planes and the W block."""
    xb = np.ascontiguousarray(x, dtype=np.float32).reshape(B, GX, MINI)
    Fx = np.fft.rfft(xb, axis=-1)  # [B, 128, 5] complex64

    planes = np.empty((P, NPL, B), dtype=_f8)
    for k, pl in _PL_RE.items():
        planes[:, pl, :] = np.clip(Fx[:, :, k].real.T, -_CLIP, _CLIP).astype(_f8)
    for k, pl in _PL_IM.items():
        planes[:, pl, :] = np.clip(Fx[:, :, k].imag.T, -_CLIP, _CLIP).astype(_f8)

    Fe = np.fft.fft(eigens.astype(np.complex64), axis=-1)  # [y, x, 8]
    M = [Fe[:, :, k].T for k in range(5)]  # M_k[x, y]
    # slot pairs in group order: Re_k -> [Wr/2, -Wi/2]; Im_k -> [Wi/2, Wr/2]
    slots = [M[0].real]
    for k, part in ((1, "re"), (1, "im"), (2, "re")):
        if part == "re":
            slots += [M[k].real, -M[k].imag]
        else:
            slots += [M[k].imag, M[k].real]
    slots += [M[4].real]
    for k, part in ((2, "im"), (3, "re"), (3, "im")):
        if part == "re":
            slots += [M[k].real, -M[k].imag]
        else:
            slots += [M[k].imag, M[k].real]
    wd = np.empty((P, NW * P), dtype=np.float32)
    for i, s in enumerate(slots):
        wd[:, i * P : (i + 1) * P] = s * _OSC
    return planes, np.ascontiguousarray(wd.astype(_bf16)).view(np.uint8)


def _host_unpack(res_list) -> np.ndarray:
    """Per-core fo planes -> full [B, OUT_CH] fp32."""
    out = np.empty((B, OUT_CH), dtype=np.float32)
    inv = 1.0 / _OSC
    for c, r in enumerate(res_list):
        # out plane order (= group order): Re0,Re1,Im1,Re2,Re4,Im2,Re3,Im3
        f = np.asarray(r["fo"]).astype(np.float32).reshape(P, NPL, BS) * inv
        Fo = np.empty((BS, GY, 5), dtype=np.complex64)
        Fo[:, :, 0] = f[:, 0].T
        Fo[:, :, 1] = (f[:, 1] + 1j * f[:, 2]).T
        Fo[:, :, 2] = (f[:, 3] + 1j * f[:, 5]).T
        Fo[:, :, 3] = (f[:, 6] + 1j * f[:, 7]).T
        Fo[:, :, 4] = f[:, 4].T
        blk = np.fft.irfft(Fo, n=MINI, axis=-1).astype(np.float32)
        out[c * BS : (c + 1) * BS] = blk.reshape(BS, OUT_CH)
    return out


def _run(x: np.ndarray, eigens: np.ndarray, trace: bool = False):
    planes, wdb = _host_pack(x, np.asarray(eigens, dtype=np.float32))
    nc = _build_nc()
    in_maps = []
    for i in range(N_CORES):
        buf = np.empty((P, _TOT), dtype=np.uint8)
        buf[:, _WA_OFF : _WA_OFF + _WA_B] = wdb[:, :_WA_B]
        buf[:, _WB_OFF : _WB_OFF + _WB_B] = wdb[:, _WA_B:]
        for pl, off in _XOFF.items():
            buf[:, off : off + BS] = planes[
                :, pl, i * BS : (i + 1) * BS
            ].view(np.uint8)
        in_maps.append({"fx": buf.view(_f8)})
    res = run_bass_kernel_spmd(nc, in_maps, list(range(N_CORES)), trace=trace)
    out = _host_unpack([res.results[i] for i in range(N_CORES)])
    return out, res


def kernel(x: np.ndarray, eigens: np.ndarray) -> np.ndarray:
    out, _ = _run(x, eigens)
    return out
